# revision 32
# baseline (speedup 1.0000x reference)
"""Multi-head cross-attention (b=2, n=m=2048, dim=1024, 16 heads) on 8 trn2 cores.

Sharding: core = be*4 + g  (be = batch element, g = head group of 4 heads).
Each core computes, for its batch element and its 4 heads:
    Q^T = (wq_g @ x1^T), K^T = (wk_g @ x2^T), V = x2 @ wv_g^T
    S^T = K^T_h-slices.T @ Q^T_h  (per head), P = exp(S * scale)  (no max
    subtraction needed, logits are ~N(0,1)), O^T = [V | 1].T @ P  (the ones
    column yields the softmax denominator for free), normalize via reciprocal
    + K=1 broadcast matmul, then y_partial = O @ wo_g^T.
Host sums the 4 head-group partials per batch element and adds the bias.

All matmuls run in float32r (TF32-like, full PE rate for free dims >= 256,
~1.5e-4 relative error). Host pre-transposes inputs so the device layout is
transpose-free. exp runs on ACT (the bottleneck engine, ~1 elem/cycle/lane)
in 1024-wide ops; head 0's S^T+exp work for the first n-chunk is emitted
inside the K/V loop so ACT starts as early as possible.
"""

import sys

if "/opt/trn_rl_repo" not in sys.path:
    sys.path.insert(0, "/opt/trn_rl_repo")

import numpy as np

import concourse.tile as tile
from concourse import bacc, mybir
from concourse import bass_utils

P = 128
NTOK = 2048            # n = m = token count per batch element
DIM = 1024
HPC = 4                # heads per core
DH = 64                # head dim
HD = HPC * DH          # 256 = per-core projection width
ECH = DIM // P         # 8 contraction chunks
NCH = NTOK // 512      # 4 n-chunks of 512
MT = NTOK // P         # 16 m-tiles of 128
SCALE = DH ** -0.5
F32 = mybir.dt.float32
F32R = mybir.dt.float32r

_CACHE: dict = {}


def _build(trace_sim: bool = False, repeat: int = 1):
    EXP = mybir.ActivationFunctionType.Exp
    nc = bacc.Bacc("TRN2", target_bir_lowering=False, debug=False, num_devices=8)
    x1T = nc.dram_tensor("x1t", [DIM, NTOK], F32R, kind="ExternalInput").ap()
    x2T = nc.dram_tensor("x2t", [DIM, NTOK], F32R, kind="ExternalInput").ap()
    wqT = nc.dram_tensor("wqt", [DIM, HD], F32R, kind="ExternalInput").ap()
    wkT = nc.dram_tensor("wkt", [DIM, HD], F32R, kind="ExternalInput").ap()
    wvT = nc.dram_tensor("wvt", [DIM, HD], F32R, kind="ExternalInput").ap()
    woT = nc.dram_tensor("wot", [HD, DIM], F32R, kind="ExternalInput").ap()
    y = nc.dram_tensor("y", [NTOK, DIM], F32, kind="ExternalOutput").ap()

    x1T_s = x1T.rearrange("(po pi) n -> pi po n", pi=P)      # [128, 8, 2048]
    x2T_s = x2T.rearrange("(po pi) n -> pi po n", pi=P)
    wqT_r = wqT.rearrange("(po pi) m -> pi po m", pi=P)      # [128, 8, 256]
    wkT_r = wkT.rearrange("(po pi) m -> pi po m", pi=P)
    wvT_r = wvT.rearrange("(po pi) m -> pi po m", pi=P)
    woT_r = woT.rearrange("(po pi) e -> pi po e", pi=P)      # [128, 2, 1024]

    with tile.TileContext(nc, trace_sim=trace_sim) as tc:
      for _rep in range(repeat):
        with (
            tc.tile_pool(name="persist", bufs=1) as persist,
            tc.tile_pool(name="ps1", bufs=4, space="PSUM") as ps1,   # [128,512]
            tc.tile_pool(name="psS", bufs=2, space="PSUM") as psSp,  # [128,1024]
            tc.tile_pool(name="xq", bufs=2) as xqpool,
        ):
            wq_sb = persist.tile([P, ECH, HD], F32R, tag="wq")
            wo_sb = persist.tile([P, 2, DIM], F32R, tag="wo")
            onesf = persist.tile([P, 64], F32, tag="onesf")
            nc.vector.memset(onesf[:], 1.0)
            ones_r = persist.tile([P, 64], F32R, tag="onesr")
            nc.vector.tensor_copy(ones_r[:], onesf[:])
            QT_sb = persist.tile([P, 2, NTOK], F32R, tag="QT")
            O_sb = persist.tile([P, 2, NTOK], F32R, tag="O")
            KT_sb = persist.tile([P, 2, NTOK], F32R, tag="KT")
            V_sb = persist.tile([P, MT, HPC, 65], F32R, tag="V")
            nc.vector.tensor_copy(
                V_sb[:, :, :, 64:65],
                onesf[:].rearrange("p (a b c) -> p a b c", a=MT, b=HPC, c=1),
            )

            def q_proj(nq):
                # Q^T projection for one n-chunk (256-wide x sub-chunks)
                for half in range(2):
                    cs = slice(nq * 512 + half * 256, nq * 512 + half * 256 + 256)
                    xq = xqpool.tile([P, ECH, 256], F32R, tag="xq")
                    for ec in range(ECH):
                        nc.sync.dma_start(xq[:, ec], x1T_s[:, ec, cs])
                    for pg in range(2):
                        psq = ps1.tile(
                            [P, 512], F32, tag="b1", name=f"psq{nq}{half}{pg}"
                        )
                        for ec in range(ECH):
                            nc.tensor.matmul(
                                psq[:, 0:256],
                                wq_sb[:, ec, pg * P:(pg + 1) * P],
                                xq[:, ec, :],
                                start=(ec == 0),
                                stop=(ec == ECH - 1),
                            )
                        nc.vector.tensor_copy(QT_sb[:, pg, cs], psq[:, 0:256])

            def s_exp_pair(nq, h, mtp, expS):
                # one [128,1024] psS pair: S^T for m-tiles (2*mtp, 2*mtp+1)
                pg, off = h // 2, 64 * (h % 2)
                ns = slice(nq * 512, (nq + 1) * 512)
                psS = psSp.tile([P, 1024], F32, tag="psS", name=f"psS{nq}{h}{mtp}")
                for sub in range(2):
                    mt = 2 * mtp + sub
                    nc.tensor.matmul(
                        psS[:, sub * 512:(sub + 1) * 512],
                        KT_sb[off:off + 64, pg, mt * P:(mt + 1) * P],
                        QT_sb[off:off + 64, pg, ns],
                        start=True,
                        stop=True,
                    )
                nc.scalar.activation(
                    expS[:, 2 * mtp:2 * mtp + 2, :].rearrange("p a b -> p (a b)"),
                    psS[:],
                    EXP,
                    scale=SCALE,
                )

            with (
                tc.tile_pool(name="wkv", bufs=1) as wkvpool,
                tc.tile_pool(name="xk", bufs=2) as xkpool,
            ):
                # weights for K first (needed earliest), per-chunk DMAs
                wk_sb = wkvpool.tile([P, ECH, HD], F32R, tag="wk")
                for ec in range(ECH):
                    nc.sync.dma_start(wk_sb[:, ec], wkT_r[:, ec])
                wv_sb = wkvpool.tile([P, ECH, HD], F32R, tag="wv")
                for ec in range(ECH):
                    nc.sync.dma_start(wv_sb[:, ec], wvT_r[:, ec])

                # ---- single x2 pass: K^T projection + V projection; h0's
                # S^T+exp for the first n-chunk is emitted as K tiles land so
                # the ACT engine (bottleneck) starts early ----
                for nq in range(NCH):
                    ns = slice(nq * 512, (nq + 1) * 512)
                    xk = xkpool.tile([P, ECH, 512], F32R, tag="xk")
                    for ec in range(ECH):
                        nc.sync.dma_start(xk[:, ec], x2T_s[:, ec, ns])
                    for pg in range(2):
                        psq = ps1.tile([P, 512], F32, tag="b1", name=f"psk{nq}{pg}")
                        for ec in range(ECH):
                            nc.tensor.matmul(
                                psq[:],
                                wk_sb[:, ec, pg * P:(pg + 1) * P],
                                xk[:, ec, :],
                                start=(ec == 0),
                                stop=(ec == ECH - 1),
                            )
                        nc.vector.tensor_copy(KT_sb[:, pg, ns], psq[:])
                    # V for the 4 m-tiles covered by this x2 chunk
                    for sub in range(4):
                        mt = 4 * nq + sub
                        pv = ps1.tile([P, 512], F32, tag="b1", name=f"psv{mt}")
                        for ec in range(ECH):
                            nc.tensor.matmul(
                                pv[:, 0:256],
                                xk[:, ec, sub * P:(sub + 1) * P],
                                wv_sb[:, ec, :],
                                start=(ec == 0),
                                stop=(ec == ECH - 1),
                            )
                        nc.vector.tensor_copy(
                            V_sb[:, mt, :, 0:64],
                            pv[:, 0:256].rearrange("p (h d) -> p h d", d=64),
                        )
                    if nq == 0:
                        nc.sync.dma_start(wq_sb[:], wqT_r)
                        q_proj(0)
                    if nq == NCH - 1:
                        nc.sync.dma_start(wo_sb[:], woT_r)

            # ---- per n-chunk: Q^T projection, attention, out-projection ----
            with (
                tc.tile_pool(name="exps", bufs=2) as expool,
                tc.tile_pool(name="rec", bufs=1) as recpool,
                tc.tile_pool(name="bcp", bufs=1) as bcpool,
                tc.tile_pool(name="otmp", bufs=1) as tmppool,
                tc.tile_pool(name="ysb", bufs=2) as ypool,
            ):
                for nq in range(NCH):
                    ns = slice(nq * 512, (nq + 1) * 512)
                    if nq + 1 < NCH:
                        q_proj(nq + 1)
                    for h in range(HPC):
                        pg, off = h // 2, 64 * (h % 2)
                        expS = expool.tile(
                            [P, MT, 512], F32R, tag="expS", name=f"expS{nq}{h}"
                        )
                        for mtp in range(MT // 2):
                            s_exp_pair(nq, h, mtp, expS)
                        psO = ps1.tile([P, 512], F32, tag="b1", name=f"psO{nq}{h}")
                        for mt in range(MT):
                            nc.tensor.matmul(
                                psO[0:65, :],
                                V_sb[:, mt, h, :],
                                expS[:, mt, :],
                                start=(mt == 0),
                                stop=(mt == MT - 1),
                            )
                        rec = recpool.tile([P, 512], F32R, tag="rec")
                        with nc.allow_low_precision(
                            reason="fp32r rounding of softmax denom reciprocal"
                        ):
                            nc.vector.reciprocal(rec[64:65, :], psO[64:65, :])
                        psB = ps1.tile([P, 512], F32, tag="b1", name=f"psB{nq}{h}")
                        nc.tensor.matmul(
                            psB[0:64, :],
                            ones_r[64:65, :],
                            rec[64:65, :],
                            start=True,
                            stop=True,
                        )
                        bc = bcpool.tile([64, 512], F32, tag="bc")
                        nc.vector.tensor_copy(bc[:], psB[0:64, :])
                        if off == 0:
                            nc.vector.tensor_mul(
                                O_sb[0:64, pg, ns], psO[0:64, :], bc[:]
                            )
                        else:
                            tmp = tmppool.tile([64, 512], F32R, tag="otmp")
                            nc.vector.tensor_mul(tmp[:], psO[0:64, :], bc[:])
                            nc.sync.dma_start(O_sb[64:128, pg, ns], tmp[:])
                    # out-projection deferred one chunk: fills PE gaps while
                    # ACT works on the next chunk's exp
                    if nq > 0:
                        out_proj(nc, nq - 1, ps1, ypool, wo_sb, O_sb, y)
                out_proj(nc, NCH - 1, ps1, ypool, wo_sb, O_sb, y)
    nc.compile()
    return nc


def out_proj(nc, nq, ps1, ypool, wo_sb, O_sb, y):
    for nt in range(4 * nq, 4 * nq + 4):
        y_sb = ypool.tile([P, DIM], F32, tag="y", name=f"ysb{nt}")
        for eo in range(2):
            psY = ps1.tile([P, 512], F32, tag="b1", name=f"psY{nt}{eo}")
            for hd in range(2):
                nc.tensor.matmul(
                    psY[:],
                    O_sb[:, hd, nt * P:(nt + 1) * P],
                    wo_sb[:, hd, eo * 512:(eo + 1) * 512],
                    start=(hd == 0),
                    stop=(hd == 1),
                )
            nc.vector.tensor_copy(y_sb[:, eo * 512:(eo + 1) * 512], psY[:])
        nc.gpsimd.dma_start(y[nt * P:(nt + 1) * P, :], y_sb[:])


def get_nc(trace_sim: bool = False, repeat: int = 1):
    key = ("nc", trace_sim, repeat)
    if key not in _CACHE:
        _CACHE[key] = _build(trace_sim, repeat)
    return _CACHE[key]


def make_in_maps(x1, x2, wq, wk, wv, wo):
    x1 = np.asarray(x1, dtype=np.float32)
    x2 = np.asarray(x2, dtype=np.float32)
    wq = np.asarray(wq, dtype=np.float32)
    wk = np.asarray(wk, dtype=np.float32)
    wv = np.asarray(wv, dtype=np.float32)
    wo = np.asarray(wo, dtype=np.float32)
    in_maps = []
    for core in range(8):
        be, g = core // 4, core % 4
        sl = slice(HD * g, HD * (g + 1))
        in_maps.append({
            "x1t": np.ascontiguousarray(x1[be].T),
            "x2t": np.ascontiguousarray(x2[be].T),
            "wqt": np.ascontiguousarray(wq[sl, :].T),
            "wkt": np.ascontiguousarray(wk[sl, :].T),
            "wvt": np.ascontiguousarray(wv[sl, :].T),
            "wot": np.ascontiguousarray(wo[:, sl].T),
        })
    return in_maps


def assemble(results, bo):
    bo = np.asarray(bo, dtype=np.float32)
    out = np.empty((2, NTOK, DIM), np.float32)
    for be in range(2):
        acc = results[be * 4]["y"].copy()
        for g in range(1, 4):
            acc += results[be * 4 + g]["y"]
        out[be] = acc + bo
    return out


def kernel(x1, x2, wq, wk, wv, wo, bo):
    nc = get_nc()
    in_maps = make_in_maps(x1, x2, wq, wk, wv, wo)
    last_err = None
    for attempt in range(3):
        try:
            res = bass_utils.run_bass_kernel_spmd(
                nc, in_maps, core_ids=list(range(8))
            )
            return assemble(res.results, bo)
        except Exception as e:  # transient NRT_EXEC_UNIT_UNRECOVERABLE etc.
            last_err = e
            import time as _time
            _time.sleep(5 * (attempt + 1))
    raise last_err


# revision 35
# speedup vs baseline: 1.1961x; 1.1961x over previous
"""Multi-head cross-attention (b=2, n=m=2048, dim=1024, 16 heads) on 8 trn2 cores.

Sharding: core = be*4 + g  (be = batch element, g = head group of 4 heads).
Each core computes, for its batch element and its 4 heads:
    Q^T = (wq_g @ x1^T), K^T = (wk_g @ x2^T), V = x2 @ wv_g^T
    S^T = K^T_h-slices.T @ Q^T_h  (per head), P = exp(S * scale)  (no max
    subtraction needed, logits are ~N(0,1)), O^T = [V | 1].T @ P  (the ones
    column yields the softmax denominator for free), normalize via reciprocal
    + K=1 broadcast matmul, then y_partial = O @ wo_g^T.
Host sums the 4 head-group partials per batch element and adds the bias.

All matmuls run in float32r (TF32-like, full PE rate for free dims >= 256,
~1.5e-4 relative error). Host pre-transposes inputs so the device layout is
transpose-free. exp runs on ACT (the bottleneck engine, ~1 elem/cycle/lane)
in 1024-wide ops; head 0's S^T+exp work for the first n-chunk is emitted
inside the K/V loop so ACT starts as early as possible.
"""

import sys

if "/opt/trn_rl_repo" not in sys.path:
    sys.path.insert(0, "/opt/trn_rl_repo")

import numpy as np

import concourse.tile as tile
from concourse import bacc, mybir
from concourse import bass_utils

P = 128
NTOK = 2048            # n = m = token count per batch element
DIM = 1024
HPC = 4                # heads per core
DH = 64                # head dim
HD = HPC * DH          # 256 = per-core projection width
ECH = DIM // P         # 8 contraction chunks
NCH = NTOK // 512      # 4 n-chunks of 512
MT = NTOK // P         # 16 m-tiles of 128
SCALE = DH ** -0.5
F32 = mybir.dt.float32
F32R = mybir.dt.float32r

_CACHE: dict = {}


def _build(trace_sim: bool = False, repeat: int = 1):
    EXP = mybir.ActivationFunctionType.Exp
    nc = bacc.Bacc("TRN2", target_bir_lowering=False, debug=False, num_devices=8)
    x1T = nc.dram_tensor("x1t", [DIM, NTOK], F32R, kind="ExternalInput").ap()
    x2T = nc.dram_tensor("x2t", [DIM, NTOK], F32R, kind="ExternalInput").ap()
    wqT = nc.dram_tensor("wqt", [DIM, HD], F32R, kind="ExternalInput").ap()
    wkT = nc.dram_tensor("wkt", [DIM, HD], F32R, kind="ExternalInput").ap()
    wvT = nc.dram_tensor("wvt", [DIM, HD], F32R, kind="ExternalInput").ap()
    woT = nc.dram_tensor("wot", [HD, DIM], F32R, kind="ExternalInput").ap()
    y = nc.dram_tensor("y", [NTOK, DIM], F32, kind="ExternalOutput").ap()

    x1T_s = x1T.rearrange("(po pi) n -> pi po n", pi=P)      # [128, 8, 2048]
    x2T_s = x2T.rearrange("(po pi) n -> pi po n", pi=P)
    wqT_r = wqT.rearrange("(po pi) m -> pi po m", pi=P)      # [128, 8, 256]
    wkT_r = wkT.rearrange("(po pi) m -> pi po m", pi=P)
    wvT_r = wvT.rearrange("(po pi) m -> pi po m", pi=P)
    woT_r = woT.rearrange("(po pi) e -> pi po e", pi=P)      # [128, 2, 1024]

    with tile.TileContext(nc, trace_sim=trace_sim) as tc:
      for _rep in range(repeat):
        with (
            tc.tile_pool(name="persist", bufs=1) as persist,
            tc.tile_pool(name="ps1", bufs=4, space="PSUM") as ps1,   # [128,512]
            tc.tile_pool(name="psS", bufs=2, space="PSUM") as psSp,  # [128,1024]
            tc.tile_pool(name="xq", bufs=2) as xqpool,
        ):
            wq_sb = persist.tile([P, ECH, HD], F32R, tag="wq")
            wo_sb = persist.tile([P, 2, DIM], F32R, tag="wo")
            onesf = persist.tile([P, 64], F32, tag="onesf")
            nc.vector.memset(onesf[:], 1.0)
            ones_r = persist.tile([P, 64], F32R, tag="onesr")
            nc.vector.tensor_copy(ones_r[:], onesf[:])
            QT_sb = persist.tile([P, 2, NTOK], F32R, tag="QT")
            O_sb = persist.tile([P, 2, NTOK], F32R, tag="O")
            KT_sb = persist.tile([P, 2, NTOK], F32R, tag="KT")
            V_sb = persist.tile([P, MT, HPC, 65], F32R, tag="V")
            nc.vector.tensor_copy(
                V_sb[:, :, :, 64:65],
                onesf[:].rearrange("p (a b c) -> p a b c", a=MT, b=HPC, c=1),
            )

            def q_proj(nq):
                # Q^T projection for one n-chunk (256-wide x sub-chunks)
                for half in range(2):
                    cs = slice(nq * 512 + half * 256, nq * 512 + half * 256 + 256)
                    xq = xqpool.tile([P, ECH, 256], F32R, tag="xq")
                    for ec in range(ECH):
                        nc.sync.dma_start(xq[:, ec], x1T_s[:, ec, cs])
                    for pg in range(2):
                        psq = ps1.tile(
                            [P, 512], F32, tag="b1", name=f"psq{nq}{half}{pg}"
                        )
                        for ec in range(ECH):
                            nc.tensor.matmul(
                                psq[:, 0:256],
                                wq_sb[:, ec, pg * P:(pg + 1) * P],
                                xq[:, ec, :],
                                start=(ec == 0),
                                stop=(ec == ECH - 1),
                            )
                        nc.vector.tensor_copy(QT_sb[:, pg, cs], psq[:, 0:256])

            def s_exp_pair(nq, h, mtp, expS):
                # one [128,1024] psS pair: S^T for m-tiles (2*mtp, 2*mtp+1)
                pg, off = h // 2, 64 * (h % 2)
                ns = slice(nq * 512, (nq + 1) * 512)
                psS = psSp.tile([P, 1024], F32, tag="psS", name=f"psS{nq}{h}{mtp}")
                for sub in range(2):
                    mt = 2 * mtp + sub
                    nc.tensor.matmul(
                        psS[:, sub * 512:(sub + 1) * 512],
                        KT_sb[off:off + 64, pg, mt * P:(mt + 1) * P],
                        QT_sb[off:off + 64, pg, ns],
                        start=True,
                        stop=True,
                    )
                nc.scalar.activation(
                    expS[:, 2 * mtp:2 * mtp + 2, :].rearrange("p a b -> p (a b)"),
                    psS[:],
                    EXP,
                    scale=SCALE,
                )

            with (
                tc.tile_pool(name="wkv", bufs=1) as wkvpool,
                tc.tile_pool(name="xk", bufs=2) as xkpool,
            ):
                # weights for K first (needed earliest), per-chunk DMAs
                wk_sb = wkvpool.tile([P, ECH, HD], F32R, tag="wk")
                for ec in range(ECH):
                    nc.sync.dma_start(wk_sb[:, ec], wkT_r[:, ec])
                wv_sb = wkvpool.tile([P, ECH, HD], F32R, tag="wv")

                # ---- single x2 pass: K^T projection + V projection; h0's
                # S^T+exp for the first n-chunk is emitted as K tiles land so
                # the ACT engine (bottleneck) starts early ----
                for nq in range(NCH):
                    ns = slice(nq * 512, (nq + 1) * 512)
                    xk = xkpool.tile([P, ECH, 512], F32R, tag="xk")
                    for ec in range(ECH):
                        nc.sync.dma_start(xk[:, ec], x2T_s[:, ec, ns])
                    if nq == 0:
                        for ec in range(ECH):
                            nc.sync.dma_start(wv_sb[:, ec], wvT_r[:, ec])
                    for pg in range(2):
                        psq = ps1.tile([P, 512], F32, tag="b1", name=f"psk{nq}{pg}")
                        for ec in range(ECH):
                            nc.tensor.matmul(
                                psq[:],
                                wk_sb[:, ec, pg * P:(pg + 1) * P],
                                xk[:, ec, :],
                                start=(ec == 0),
                                stop=(ec == ECH - 1),
                            )
                        nc.vector.tensor_copy(KT_sb[:, pg, ns], psq[:])
                    # V for the 4 m-tiles covered by this x2 chunk
                    for sub in range(4):
                        mt = 4 * nq + sub
                        pv = ps1.tile([P, 512], F32, tag="b1", name=f"psv{mt}")
                        for ec in range(ECH):
                            nc.tensor.matmul(
                                pv[:, 0:256],
                                xk[:, ec, sub * P:(sub + 1) * P],
                                wv_sb[:, ec, :],
                                start=(ec == 0),
                                stop=(ec == ECH - 1),
                            )
                        nc.vector.tensor_copy(
                            V_sb[:, mt, :, 0:64],
                            pv[:, 0:256].rearrange("p (h d) -> p h d", d=64),
                        )
                    if nq == 0:
                        nc.sync.dma_start(wq_sb[:], wqT_r)
                        q_proj(0)
                    if nq == NCH - 1:
                        nc.sync.dma_start(wo_sb[:], woT_r)

            # ---- per n-chunk: Q^T projection, attention, out-projection ----
            with (
                tc.tile_pool(name="exps", bufs=2) as expool,
                tc.tile_pool(name="rec", bufs=1) as recpool,
                tc.tile_pool(name="bcp", bufs=1) as bcpool,
                tc.tile_pool(name="otmp", bufs=1) as tmppool,
                tc.tile_pool(name="ysb", bufs=2) as ypool,
            ):
                for nq in range(NCH):
                    ns = slice(nq * 512, (nq + 1) * 512)
                    if nq + 1 < NCH:
                        q_proj(nq + 1)
                    for h in range(HPC):
                        pg, off = h // 2, 64 * (h % 2)
                        expS = expool.tile(
                            [P, MT, 512], F32R, tag="expS", name=f"expS{nq}{h}"
                        )
                        for mtp in range(MT // 2):
                            s_exp_pair(nq, h, mtp, expS)
                        psO = ps1.tile([P, 512], F32, tag="b1", name=f"psO{nq}{h}")
                        for mt in range(MT):
                            nc.tensor.matmul(
                                psO[0:65, :],
                                V_sb[:, mt, h, :],
                                expS[:, mt, :],
                                start=(mt == 0),
                                stop=(mt == MT - 1),
                            )
                        rec = recpool.tile([P, 512], F32R, tag="rec")
                        with nc.allow_low_precision(
                            reason="fp32r rounding of softmax denom reciprocal"
                        ):
                            nc.vector.reciprocal(rec[64:65, :], psO[64:65, :])
                        nc.sync.dma_start(rec[0:1, :], rec[64:65, :])
                        bc = bcpool.tile([64, 512], F32R, tag="bc")
                        nc.gpsimd.partition_broadcast(bc[:], rec[0:1, :])
                        if off == 0:
                            nc.vector.tensor_mul(
                                O_sb[0:64, pg, ns], psO[0:64, :], bc[:]
                            )
                        else:
                            tmp = tmppool.tile([64, 512], F32R, tag="otmp")
                            nc.vector.tensor_mul(tmp[:], psO[0:64, :], bc[:])
                            nc.sync.dma_start(O_sb[64:128, pg, ns], tmp[:])
                    # out-projection deferred one chunk: fills PE gaps while
                    # ACT works on the next chunk's exp
                    if nq > 0:
                        out_proj(nc, nq - 1, ps1, ypool, wo_sb, O_sb, y)
                out_proj(nc, NCH - 1, ps1, ypool, wo_sb, O_sb, y)
    nc.compile()
    return nc


def out_proj(nc, nq, ps1, ypool, wo_sb, O_sb, y):
    for nt in range(4 * nq, 4 * nq + 4):
        y_sb = ypool.tile([P, DIM], F32, tag="y", name=f"ysb{nt}")
        for eo in range(2):
            psY = ps1.tile([P, 512], F32, tag="b1", name=f"psY{nt}{eo}")
            for hd in range(2):
                nc.tensor.matmul(
                    psY[:],
                    O_sb[:, hd, nt * P:(nt + 1) * P],
                    wo_sb[:, hd, eo * 512:(eo + 1) * 512],
                    start=(hd == 0),
                    stop=(hd == 1),
                )
            nc.vector.tensor_copy(y_sb[:, eo * 512:(eo + 1) * 512], psY[:])
        nc.gpsimd.dma_start(y[nt * P:(nt + 1) * P, :], y_sb[:])


def get_nc(trace_sim: bool = False, repeat: int = 1):
    key = ("nc", trace_sim, repeat)
    if key not in _CACHE:
        _CACHE[key] = _build(trace_sim, repeat)
    return _CACHE[key]


def make_in_maps(x1, x2, wq, wk, wv, wo):
    x1 = np.asarray(x1, dtype=np.float32)
    x2 = np.asarray(x2, dtype=np.float32)
    wq = np.asarray(wq, dtype=np.float32)
    wk = np.asarray(wk, dtype=np.float32)
    wv = np.asarray(wv, dtype=np.float32)
    wo = np.asarray(wo, dtype=np.float32)
    in_maps = []
    for core in range(8):
        be, g = core // 4, core % 4
        sl = slice(HD * g, HD * (g + 1))
        in_maps.append({
            "x1t": np.ascontiguousarray(x1[be].T),
            "x2t": np.ascontiguousarray(x2[be].T),
            "wqt": np.ascontiguousarray(wq[sl, :].T),
            "wkt": np.ascontiguousarray(wk[sl, :].T),
            "wvt": np.ascontiguousarray(wv[sl, :].T),
            "wot": np.ascontiguousarray(wo[:, sl].T),
        })
    return in_maps


def assemble(results, bo):
    bo = np.asarray(bo, dtype=np.float32)
    out = np.empty((2, NTOK, DIM), np.float32)
    for be in range(2):
        acc = results[be * 4]["y"].copy()
        for g in range(1, 4):
            acc += results[be * 4 + g]["y"]
        out[be] = acc + bo
    return out


def kernel(x1, x2, wq, wk, wv, wo, bo):
    nc = get_nc()
    in_maps = make_in_maps(x1, x2, wq, wk, wv, wo)
    last_err = None
    for attempt in range(3):
        try:
            res = bass_utils.run_bass_kernel_spmd(
                nc, in_maps, core_ids=list(range(8))
            )
            return assemble(res.results, bo)
        except Exception as e:  # transient NRT_EXEC_UNIT_UNRECOVERABLE etc.
            last_err = e
            import time as _time
            _time.sleep(5 * (attempt + 1))
    raise last_err


# revision 36
# speedup vs baseline: 1.3502x; 1.1289x over previous
"""Multi-head cross-attention (b=2, n=m=2048, dim=1024, 16 heads) on 8 trn2 cores.

Sharding: core = be*4 + g  (be = batch element, g = head group of 4 heads).
Each core computes, for its batch element and its 4 heads:
    Q^T = (wq_g @ x1^T), K^T = (wk_g @ x2^T), V = x2 @ wv_g^T
    S^T = K^T_h-slices.T @ Q^T_h  (per head), P = exp(S * scale)  (no max
    subtraction needed, logits are ~N(0,1)), O^T = [V | 1].T @ P  (the ones
    column yields the softmax denominator for free), normalize via reciprocal
    + K=1 broadcast matmul, then y_partial = O @ wo_g^T.
Host sums the 4 head-group partials per batch element and adds the bias.

All matmuls run in float32r (TF32-like, full PE rate for free dims >= 256,
~1.5e-4 relative error). Host pre-transposes inputs so the device layout is
transpose-free. exp runs on ACT (the bottleneck engine, ~1 elem/cycle/lane)
in 1024-wide ops; head 0's S^T+exp work for the first n-chunk is emitted
inside the K/V loop so ACT starts as early as possible.
"""

import sys

if "/opt/trn_rl_repo" not in sys.path:
    sys.path.insert(0, "/opt/trn_rl_repo")

import numpy as np

import concourse.tile as tile
from concourse import bacc, mybir
from concourse import bass_utils

P = 128
NTOK = 2048            # n = m = token count per batch element
DIM = 1024
HPC = 4                # heads per core
DH = 64                # head dim
HD = HPC * DH          # 256 = per-core projection width
ECH = DIM // P         # 8 contraction chunks
NCH = NTOK // 512      # 4 n-chunks of 512
MT = NTOK // P         # 16 m-tiles of 128
SCALE = DH ** -0.5
F32 = mybir.dt.float32
F32R = mybir.dt.float32r

_CACHE: dict = {}


def _build(trace_sim: bool = False, repeat: int = 1):
    EXP = mybir.ActivationFunctionType.Exp
    nc = bacc.Bacc("TRN2", target_bir_lowering=False, debug=False, num_devices=8)
    x1T = nc.dram_tensor("x1t", [DIM, NTOK], F32R, kind="ExternalInput").ap()
    x2T = nc.dram_tensor("x2t", [DIM, NTOK], F32R, kind="ExternalInput").ap()
    wqT = nc.dram_tensor("wqt", [DIM, HD], F32R, kind="ExternalInput").ap()
    wkT = nc.dram_tensor("wkt", [DIM, HD], F32R, kind="ExternalInput").ap()
    wvT = nc.dram_tensor("wvt", [DIM, HD], F32R, kind="ExternalInput").ap()
    woT = nc.dram_tensor("wot", [HD, DIM], F32R, kind="ExternalInput").ap()
    y = nc.dram_tensor("y", [NTOK, DIM], F32, kind="ExternalOutput").ap()

    x1T_s = x1T.rearrange("(po pi) n -> pi po n", pi=P)      # [128, 8, 2048]
    x2T_s = x2T.rearrange("(po pi) n -> pi po n", pi=P)
    wqT_r = wqT.rearrange("(po pi) m -> pi po m", pi=P)      # [128, 8, 256]
    wkT_r = wkT.rearrange("(po pi) m -> pi po m", pi=P)
    wvT_r = wvT.rearrange("(po pi) m -> pi po m", pi=P)
    woT_r = woT.rearrange("(po pi) e -> pi po e", pi=P)      # [128, 2, 1024]

    with tile.TileContext(nc, trace_sim=trace_sim) as tc:
      for _rep in range(repeat):
        with (
            tc.tile_pool(name="persist", bufs=1) as persist,
            tc.tile_pool(name="ps1", bufs=4, space="PSUM") as ps1,   # [128,512]
            tc.tile_pool(name="psS", bufs=2, space="PSUM") as psSp,  # [128,1024]
            tc.tile_pool(name="xq", bufs=2) as xqpool,
        ):
            wq_sb = persist.tile([P, ECH, HD], F32R, tag="wq")
            wo_sb = persist.tile([P, 2, DIM], F32R, tag="wo")
            onesf = persist.tile([P, 64], F32, tag="onesf")
            nc.vector.memset(onesf[:], 1.0)
            QT_sb = persist.tile([P, 2, NTOK], F32R, tag="QT")
            O_sb = persist.tile([P, 2, NTOK], F32R, tag="O")
            KT_sb = persist.tile([P, 2, NTOK], F32R, tag="KT")
            V_sb = persist.tile([P, MT, HPC, 65], F32R, tag="V")
            nc.vector.tensor_copy(
                V_sb[:, :, :, 64:65],
                onesf[:].rearrange("p (a b c) -> p a b c", a=MT, b=HPC, c=1),
            )

            def q_proj(nq):
                # Q^T projection for one n-chunk (256-wide x sub-chunks)
                for half in range(2):
                    cs = slice(nq * 512 + half * 256, nq * 512 + half * 256 + 256)
                    xq = xqpool.tile([P, ECH, 256], F32R, tag="xq")
                    for ec in range(ECH):
                        nc.sync.dma_start(xq[:, ec], x1T_s[:, ec, cs])
                    for pg in range(2):
                        psq = ps1.tile(
                            [P, 512], F32, tag="b1", name=f"psq{nq}{half}{pg}"
                        )
                        for ec in range(ECH):
                            nc.tensor.matmul(
                                psq[:, 0:256],
                                wq_sb[:, ec, pg * P:(pg + 1) * P],
                                xq[:, ec, :],
                                start=(ec == 0),
                                stop=(ec == ECH - 1),
                            )
                        nc.vector.tensor_copy(QT_sb[:, pg, cs], psq[:, 0:256])

            def s_exp_pair(nq, h, mtp, expS):
                # one [128,1024] psS pair: S^T for m-tiles (2*mtp, 2*mtp+1)
                pg, off = h // 2, 64 * (h % 2)
                ns = slice(nq * 512, (nq + 1) * 512)
                psS = psSp.tile([P, 1024], F32, tag="psS", name=f"psS{nq}{h}{mtp}")
                for sub in range(2):
                    mt = 2 * mtp + sub
                    nc.tensor.matmul(
                        psS[:, sub * 512:(sub + 1) * 512],
                        KT_sb[off:off + 64, pg, mt * P:(mt + 1) * P],
                        QT_sb[off:off + 64, pg, ns],
                        start=True,
                        stop=True,
                    )
                nc.scalar.activation(
                    expS[:, 2 * mtp:2 * mtp + 2, :].rearrange("p a b -> p (a b)"),
                    psS[:],
                    EXP,
                    scale=SCALE,
                )

            with (
                tc.tile_pool(name="wkv", bufs=1) as wkvpool,
                tc.tile_pool(name="xk", bufs=2) as xkpool,
            ):
                # weights for K first (needed earliest), per-chunk DMAs
                wk_sb = wkvpool.tile([P, ECH, HD], F32R, tag="wk")
                for ec in range(ECH):
                    nc.sync.dma_start(wk_sb[:, ec], wkT_r[:, ec])
                wv_sb = wkvpool.tile([P, ECH, HD], F32R, tag="wv")

                # ---- single x2 pass: K^T projection + V projection; h0's
                # S^T+exp for the first n-chunk is emitted as K tiles land so
                # the ACT engine (bottleneck) starts early ----
                for nq in range(NCH):
                    ns = slice(nq * 512, (nq + 1) * 512)
                    xk = xkpool.tile([P, ECH, 512], F32R, tag="xk")
                    for ec in range(ECH):
                        nc.sync.dma_start(xk[:, ec], x2T_s[:, ec, ns])
                    if nq == 0:
                        for ec in range(ECH):
                            nc.sync.dma_start(wv_sb[:, ec], wvT_r[:, ec])
                    for pg in range(2):
                        psq = ps1.tile([P, 512], F32, tag="b1", name=f"psk{nq}{pg}")
                        for ec in range(ECH):
                            nc.tensor.matmul(
                                psq[:],
                                wk_sb[:, ec, pg * P:(pg + 1) * P],
                                xk[:, ec, :],
                                start=(ec == 0),
                                stop=(ec == ECH - 1),
                            )
                        nc.vector.tensor_copy(KT_sb[:, pg, ns], psq[:])
                    # V for the 4 m-tiles covered by this x2 chunk
                    for sub in range(4):
                        mt = 4 * nq + sub
                        pv = ps1.tile([P, 512], F32, tag="b1", name=f"psv{mt}")
                        for ec in range(ECH):
                            nc.tensor.matmul(
                                pv[:, 0:256],
                                xk[:, ec, sub * P:(sub + 1) * P],
                                wv_sb[:, ec, :],
                                start=(ec == 0),
                                stop=(ec == ECH - 1),
                            )
                        nc.vector.tensor_copy(
                            V_sb[:, mt, :, 0:64],
                            pv[:, 0:256].rearrange("p (h d) -> p h d", d=64),
                        )
                    if nq == 0:
                        nc.sync.dma_start(wq_sb[:], wqT_r)
                        q_proj(0)
                    if nq == NCH - 1:
                        nc.sync.dma_start(wo_sb[:], woT_r)

            # ---- per n-chunk: Q^T projection, attention, out-projection ----
            with (
                tc.tile_pool(name="exps", bufs=2) as expool,
                tc.tile_pool(name="rec", bufs=1) as recpool,
                tc.tile_pool(name="bcp", bufs=1) as bcpool,
                tc.tile_pool(name="otmp", bufs=1) as tmppool,
                tc.tile_pool(name="ysb", bufs=2) as ypool,
            ):
                for nq in range(NCH):
                    ns = slice(nq * 512, (nq + 1) * 512)
                    if nq + 1 < NCH:
                        q_proj(nq + 1)
                    for h in range(HPC):
                        pg, off = h // 2, 64 * (h % 2)
                        expS = expool.tile(
                            [P, MT, 512], F32R, tag="expS", name=f"expS{nq}{h}"
                        )
                        for mtp in range(MT // 2):
                            s_exp_pair(nq, h, mtp, expS)
                        psO = ps1.tile([P, 512], F32, tag="b1", name=f"psO{nq}{h}")
                        for mt in range(MT):
                            nc.tensor.matmul(
                                psO[0:65, :],
                                V_sb[:, mt, h, :],
                                expS[:, mt, :],
                                start=(mt == 0),
                                stop=(mt == MT - 1),
                            )
                        rec = recpool.tile([P, 512], F32R, tag="rec")
                        with nc.allow_low_precision(
                            reason="fp32r rounding of softmax denom reciprocal"
                        ):
                            nc.vector.reciprocal(rec[64:65, :], psO[64:65, :])
                        nc.sync.dma_start(rec[0:1, :], rec[64:65, :])
                        bc = bcpool.tile([64, 512], F32R, tag="bc")
                        nc.gpsimd.partition_broadcast(bc[:], rec[0:1, :])
                        if off == 0:
                            nc.vector.tensor_mul(
                                O_sb[0:64, pg, ns], psO[0:64, :], bc[:]
                            )
                        else:
                            tmp = tmppool.tile([64, 512], F32R, tag="otmp")
                            nc.vector.tensor_mul(tmp[:], psO[0:64, :], bc[:])
                            nc.sync.dma_start(O_sb[64:128, pg, ns], tmp[:])
                    # out-projection deferred one chunk: fills PE gaps while
                    # ACT works on the next chunk's exp
                    if nq > 0:
                        out_proj(nc, nq - 1, ps1, ypool, wo_sb, O_sb, y)
                out_proj(nc, NCH - 1, ps1, ypool, wo_sb, O_sb, y)
    nc.compile()
    return nc


def out_proj(nc, nq, ps1, ypool, wo_sb, O_sb, y):
    for nt in range(4 * nq, 4 * nq + 4):
        y_sb = ypool.tile([P, DIM], F32, tag="y", name=f"ysb{nt}")
        for eo in range(2):
            psY = ps1.tile([P, 512], F32, tag="b1", name=f"psY{nt}{eo}")
            for hd in range(2):
                nc.tensor.matmul(
                    psY[:],
                    O_sb[:, hd, nt * P:(nt + 1) * P],
                    wo_sb[:, hd, eo * 512:(eo + 1) * 512],
                    start=(hd == 0),
                    stop=(hd == 1),
                )
            nc.vector.tensor_copy(y_sb[:, eo * 512:(eo + 1) * 512], psY[:])
        nc.gpsimd.dma_start(y[nt * P:(nt + 1) * P, :], y_sb[:])


def get_nc(trace_sim: bool = False, repeat: int = 1):
    key = ("nc", trace_sim, repeat)
    if key not in _CACHE:
        _CACHE[key] = _build(trace_sim, repeat)
    return _CACHE[key]


def make_in_maps(x1, x2, wq, wk, wv, wo):
    x1 = np.asarray(x1, dtype=np.float32)
    x2 = np.asarray(x2, dtype=np.float32)
    wq = np.asarray(wq, dtype=np.float32)
    wk = np.asarray(wk, dtype=np.float32)
    wv = np.asarray(wv, dtype=np.float32)
    wo = np.asarray(wo, dtype=np.float32)
    in_maps = []
    for core in range(8):
        be, g = core // 4, core % 4
        sl = slice(HD * g, HD * (g + 1))
        in_maps.append({
            "x1t": np.ascontiguousarray(x1[be].T),
            "x2t": np.ascontiguousarray(x2[be].T),
            "wqt": np.ascontiguousarray(wq[sl, :].T),
            "wkt": np.ascontiguousarray(wk[sl, :].T),
            "wvt": np.ascontiguousarray(wv[sl, :].T),
            "wot": np.ascontiguousarray(wo[:, sl].T),
        })
    return in_maps


def assemble(results, bo):
    bo = np.asarray(bo, dtype=np.float32)
    out = np.empty((2, NTOK, DIM), np.float32)
    for be in range(2):
        acc = results[be * 4]["y"].copy()
        for g in range(1, 4):
            acc += results[be * 4 + g]["y"]
        out[be] = acc + bo
    return out


def kernel(x1, x2, wq, wk, wv, wo, bo):
    nc = get_nc()
    in_maps = make_in_maps(x1, x2, wq, wk, wv, wo)
    last_err = None
    for attempt in range(3):
        try:
            res = bass_utils.run_bass_kernel_spmd(
                nc, in_maps, core_ids=list(range(8))
            )
            return assemble(res.results, bo)
        except Exception as e:  # transient NRT_EXEC_UNIT_UNRECOVERABLE etc.
            last_err = e
            import time as _time
            _time.sleep(5 * (attempt + 1))
    raise last_err


# revision 38
# speedup vs baseline: 1.6941x; 1.2548x over previous
"""Multi-head cross-attention (b=2, n=m=2048, dim=1024, 16 heads) on 8 trn2 cores.

Sharding: core = be*4 + g  (be = batch element, g = head group of 4 heads).
Each core computes, for its batch element and its 4 heads:
    Q^T = (wq_g @ x1^T), K^T = (wk_g @ x2^T), V = x2 @ wv_g^T
    S^T = K^T_h-slices.T @ Q^T_h  (per head), P = exp(S * scale)  (no max
    subtraction needed, logits are ~N(0,1)), O^T = [V | 1].T @ P  (the ones
    column yields the softmax denominator for free), normalize via reciprocal
    + K=1 broadcast matmul, then y_partial = O @ wo_g^T.
Host sums the 4 head-group partials per batch element and adds the bias.

All matmuls run in float32r (TF32-like, full PE rate for free dims >= 256,
~1.5e-4 relative error). Host pre-transposes inputs so the device layout is
transpose-free. exp runs on ACT (the bottleneck engine, ~1 elem/cycle/lane)
in 1024-wide ops; head 0's S^T+exp work for the first n-chunk is emitted
inside the K/V loop so ACT starts as early as possible.
"""

import sys

if "/opt/trn_rl_repo" not in sys.path:
    sys.path.insert(0, "/opt/trn_rl_repo")

import numpy as np

import concourse.tile as tile
from concourse import bacc, mybir
from concourse import bass_utils

P = 128
NTOK = 2048            # n = m = token count per batch element
DIM = 1024
HPC = 4                # heads per core
DH = 64                # head dim
HD = HPC * DH          # 256 = per-core projection width
ECH = DIM // P         # 8 contraction chunks
NCH = NTOK // 512      # 4 n-chunks of 512
MT = NTOK // P         # 16 m-tiles of 128
SCALE = DH ** -0.5
F32 = mybir.dt.float32
F32R = mybir.dt.float32r

_CACHE: dict = {}


def _build(trace_sim: bool = False, repeat: int = 1):
    EXP = mybir.ActivationFunctionType.Exp
    nc = bacc.Bacc("TRN2", target_bir_lowering=False, debug=False, num_devices=8)
    x1T = nc.dram_tensor("x1t", [DIM, NTOK], F32R, kind="ExternalInput").ap()
    x2T = nc.dram_tensor("x2t", [DIM, NTOK], F32R, kind="ExternalInput").ap()
    wqT = nc.dram_tensor("wqt", [DIM, HD], F32R, kind="ExternalInput").ap()
    wkT = nc.dram_tensor("wkt", [DIM, HD], F32R, kind="ExternalInput").ap()
    wvT = nc.dram_tensor("wvt", [DIM, HD], F32R, kind="ExternalInput").ap()
    woT = nc.dram_tensor("wot", [HD, DIM], F32R, kind="ExternalInput").ap()
    y = nc.dram_tensor("y", [NTOK, DIM], F32, kind="ExternalOutput").ap()

    x1T_s = x1T.rearrange("(po pi) n -> pi po n", pi=P)      # [128, 8, 2048]
    x2T_s = x2T.rearrange("(po pi) n -> pi po n", pi=P)
    wqT_r = wqT.rearrange("(po pi) m -> pi po m", pi=P)      # [128, 8, 256]
    wkT_r = wkT.rearrange("(po pi) m -> pi po m", pi=P)
    wvT_r = wvT.rearrange("(po pi) m -> pi po m", pi=P)
    woT_r = woT.rearrange("(po pi) e -> pi po e", pi=P)      # [128, 2, 1024]

    with tile.TileContext(nc, trace_sim=trace_sim) as tc:
      for _rep in range(repeat):
        with (
            tc.tile_pool(name="persist", bufs=1) as persist,
            tc.tile_pool(name="ps1", bufs=4, space="PSUM") as ps1,   # [128,512]
            tc.tile_pool(name="psS", bufs=2, space="PSUM") as psSp,  # [128,1024]
            tc.tile_pool(name="xq", bufs=2) as xqpool,
        ):
            wq_sb = persist.tile([P, ECH, HD], F32R, tag="wq")
            wo_sb = persist.tile([P, 2, DIM], F32R, tag="wo")
            onesf = persist.tile([P, 64], F32, tag="onesf")
            nc.vector.memset(onesf[:], 1.0)
            QT_sb = persist.tile([P, 2, NTOK], F32R, tag="QT")
            O_sb = persist.tile([P, 2, NTOK], F32R, tag="O")
            KT_sb = persist.tile([P, 2, NTOK], F32R, tag="KT")
            V_sb = persist.tile([P, MT, HPC, 65], F32R, tag="V")
            nc.vector.tensor_copy(
                V_sb[:, :, :, 64:65],
                onesf[:].rearrange("p (a b c) -> p a b c", a=MT, b=HPC, c=1),
            )

            def q_proj(nq):
                # Q^T projection for one n-chunk (256-wide x sub-chunks)
                for half in range(2):
                    cs = slice(nq * 512 + half * 256, nq * 512 + half * 256 + 256)
                    xq = xqpool.tile([P, ECH, 256], F32R, tag="xq")
                    for ec in range(ECH):
                        nc.sync.dma_start(xq[:, ec], x1T_s[:, ec, cs])
                    for pg in range(2):
                        psq = ps1.tile(
                            [P, 512], F32, tag="b1", name=f"psq{nq}{half}{pg}"
                        )
                        for ec in range(ECH):
                            nc.tensor.matmul(
                                psq[:, 0:256],
                                wq_sb[:, ec, pg * P:(pg + 1) * P],
                                xq[:, ec, :],
                                start=(ec == 0),
                                stop=(ec == ECH - 1),
                            )
                        nc.vector.tensor_copy(QT_sb[:, pg, cs], psq[:, 0:256])

            def s_exp_pair(nq, h, mtp, expS):
                # one [128,1024] psS pair: S^T for m-tiles (2*mtp, 2*mtp+1)
                pg, off = h // 2, 64 * (h % 2)
                ns = slice(nq * 512, (nq + 1) * 512)
                psS = psSp.tile([P, 1024], F32, tag="psS", name=f"psS{nq}{h}{mtp}")
                for sub in range(2):
                    mt = 2 * mtp + sub
                    nc.tensor.matmul(
                        psS[:, sub * 512:(sub + 1) * 512],
                        KT_sb[off:off + 64, pg, mt * P:(mt + 1) * P],
                        QT_sb[off:off + 64, pg, ns],
                        start=True,
                        stop=True,
                    )
                nc.scalar.activation(
                    expS[:, 2 * mtp:2 * mtp + 2, :].rearrange("p a b -> p (a b)"),
                    psS[:],
                    EXP,
                    scale=SCALE,
                )

            with (
                tc.tile_pool(name="wkv", bufs=1) as wkvpool,
                tc.tile_pool(name="xk", bufs=2) as xkpool,
            ):
                # weights for K first (needed earliest), per-chunk DMAs
                wk_sb = wkvpool.tile([P, ECH, HD], F32R, tag="wk")
                for ec in range(ECH):
                    nc.sync.dma_start(wk_sb[:, ec], wkT_r[:, ec])
                wv_sb = wkvpool.tile([P, ECH, HD], F32R, tag="wv")

                # ---- single x2 pass: K^T projection + V projection; h0's
                # S^T+exp for the first n-chunk is emitted as K tiles land so
                # the ACT engine (bottleneck) starts early ----
                for nq in range(NCH):
                    ns = slice(nq * 512, (nq + 1) * 512)
                    xk = xkpool.tile([P, ECH, 512], F32R, tag="xk")
                    for ec in range(ECH):
                        nc.sync.dma_start(xk[:, ec], x2T_s[:, ec, ns])
                    if nq == 0:
                        for ec in range(ECH):
                            nc.sync.dma_start(wv_sb[:, ec], wvT_r[:, ec])
                    for pg in range(2):
                        psq = ps1.tile([P, 512], F32, tag="b1", name=f"psk{nq}{pg}")
                        for ec in range(ECH):
                            nc.tensor.matmul(
                                psq[:],
                                wk_sb[:, ec, pg * P:(pg + 1) * P],
                                xk[:, ec, :],
                                start=(ec == 0),
                                stop=(ec == ECH - 1),
                            )
                        nc.vector.tensor_copy(KT_sb[:, pg, ns], psq[:])
                    # V for the 4 m-tiles covered by this x2 chunk
                    for sub in range(4):
                        mt = 4 * nq + sub
                        pv = ps1.tile([P, 512], F32, tag="b1", name=f"psv{mt}")
                        for ec in range(ECH):
                            nc.tensor.matmul(
                                pv[:, 0:256],
                                xk[:, ec, sub * P:(sub + 1) * P],
                                wv_sb[:, ec, :],
                                start=(ec == 0),
                                stop=(ec == ECH - 1),
                            )
                        nc.vector.tensor_copy(
                            V_sb[:, mt, :, 0:64],
                            pv[:, 0:256].rearrange("p (h d) -> p h d", d=64),
                        )
                    if nq == 0:
                        nc.sync.dma_start(wq_sb[:], wqT_r)
                        q_proj(0)
                    if nq == NCH - 1:
                        nc.sync.dma_start(wo_sb[:], woT_r)

            # ---- per n-chunk: Q^T projection, attention, out-projection ----
            with (
                tc.tile_pool(name="exps", bufs=2) as expool,
                tc.tile_pool(name="rec", bufs=1) as recpool,
                tc.tile_pool(name="bcp", bufs=1) as bcpool,
                tc.tile_pool(name="otmp", bufs=1) as tmppool,
                tc.tile_pool(name="ysb", bufs=2) as ypool,
            ):
                for nq in range(NCH):
                    ns = slice(nq * 512, (nq + 1) * 512)
                    if nq + 1 < NCH:
                        q_proj(nq + 1)
                    for h in range(HPC):
                        pg, off = h // 2, 64 * (h % 2)
                        expS = expool.tile(
                            [P, MT, 512], F32R, tag="expS", name=f"expS{nq}{h}"
                        )
                        for mtp in range(MT // 2):
                            s_exp_pair(nq, h, mtp, expS)
                        psO = ps1.tile([P, 512], F32, tag="b1", name=f"psO{nq}{h}")
                        for mt in range(MT):
                            nc.tensor.matmul(
                                psO[0:65, :],
                                V_sb[:, mt, h, :],
                                expS[:, mt, :],
                                start=(mt == 0),
                                stop=(mt == MT - 1),
                            )
                        rec = recpool.tile([P, 512], F32R, tag="rec")
                        with nc.allow_low_precision(
                            reason="fp32r rounding of softmax denom reciprocal"
                        ):
                            nc.vector.reciprocal(rec[64:65, :], psO[64:65, :])
                        nc.sync.dma_start(rec[0:1, :], rec[64:65, :])
                        bc = bcpool.tile([64, 512], F32R, tag="bc")
                        nc.gpsimd.partition_broadcast(bc[:], rec[0:1, :])
                        if off == 0:
                            nc.vector.tensor_mul(
                                O_sb[0:64, pg, ns], psO[0:64, :], bc[:]
                            )
                        else:
                            tmp = tmppool.tile([64, 512], F32R, tag="otmp")
                            nc.vector.tensor_mul(tmp[:], psO[0:64, :], bc[:])
                            nc.sync.dma_start(O_sb[64:128, pg, ns], tmp[:])
                    # out-projection deferred one chunk: fills PE gaps while
                    # ACT works on the next chunk's exp
                    if nq > 0:
                        out_proj(nc, nq - 1, ps1, ypool, wo_sb, O_sb, y)
                out_proj(nc, NCH - 1, ps1, ypool, wo_sb, O_sb, y)
    nc.compile()
    return nc


def out_proj(nc, nq, ps1, ypool, wo_sb, O_sb, y):
    for nt in range(4 * nq, 4 * nq + 4):
        y_sb = ypool.tile([P, DIM], F32, tag="y", name=f"ysb{nt}")
        for eo in range(2):
            psY = ps1.tile([P, 512], F32, tag="b1", name=f"psY{nt}{eo}")
            for hd in range(2):
                nc.tensor.matmul(
                    psY[:],
                    O_sb[:, hd, nt * P:(nt + 1) * P],
                    wo_sb[:, hd, eo * 512:(eo + 1) * 512],
                    start=(hd == 0),
                    stop=(hd == 1),
                )
            nc.vector.tensor_copy(y_sb[:, eo * 512:(eo + 1) * 512], psY[:])
        nc.gpsimd.dma_start(y[nt * P:(nt + 1) * P, :], y_sb[:])


def get_nc(trace_sim: bool = False, repeat: int = 1):
    key = ("nc", trace_sim, repeat)
    if key not in _CACHE:
        _CACHE[key] = _build(trace_sim, repeat)
    return _CACHE[key]


def make_in_maps(x1, x2, wq, wk, wv, wo):
    x1 = np.asarray(x1, dtype=np.float32)
    x2 = np.asarray(x2, dtype=np.float32)
    wq = np.asarray(wq, dtype=np.float32)
    wk = np.asarray(wk, dtype=np.float32)
    wv = np.asarray(wv, dtype=np.float32)
    wo = np.asarray(wo, dtype=np.float32)
    in_maps = []
    for core in range(8):
        be, g = core // 4, core % 4
        sl = slice(HD * g, HD * (g + 1))
        in_maps.append({
            "x1t": np.ascontiguousarray(x1[be].T),
            "x2t": np.ascontiguousarray(x2[be].T),
            "wqt": np.ascontiguousarray(wq[sl, :].T),
            "wkt": np.ascontiguousarray(wk[sl, :].T),
            "wvt": np.ascontiguousarray(wv[sl, :].T),
            "wot": np.ascontiguousarray(wo[:, sl].T),
        })
    return in_maps


def assemble(results, bo):
    bo = np.asarray(bo, dtype=np.float32)
    out = np.empty((2, NTOK, DIM), np.float32)
    for be in range(2):
        acc = results[be * 4]["y"].copy()
        for g in range(1, 4):
            acc += results[be * 4 + g]["y"]
        out[be] = acc + bo
    return out


def kernel(x1, x2, wq, wk, wv, wo, bo):
    nc = get_nc()
    in_maps = make_in_maps(x1, x2, wq, wk, wv, wo)
    last_err = None
    for attempt in range(3):
        try:
            res = bass_utils.run_bass_kernel_spmd(
                nc, in_maps, core_ids=list(range(8))
            )
            return assemble(res.results, bo)
        except Exception as e:  # transient NRT_EXEC_UNIT_UNRECOVERABLE etc.
            last_err = e
            import time as _time
            _time.sleep(5 * (attempt + 1))
    raise last_err


# revision 40
# speedup vs baseline: 2.2397x; 1.3220x over previous
"""Multi-head cross-attention (b=2, n=m=2048, dim=1024, 16 heads) on 8 trn2 cores.

Sharding: core = be*4 + g  (be = batch element, g = head group of 4 heads).
Each core computes, for its batch element and its 4 heads:
    Q^T = (wq_g @ x1^T), K^T = (wk_g @ x2^T), V = x2 @ wv_g^T
    S^T = K^T_h-slices.T @ Q^T_h  (per head), P = exp(S * scale)  (no max
    subtraction needed, logits are ~N(0,1)), O^T = [V | 1].T @ P  (the ones
    column yields the softmax denominator for free), normalize via reciprocal
    + K=1 broadcast matmul, then y_partial = O @ wo_g^T.
Host sums the 4 head-group partials per batch element and adds the bias.

All matmuls run in float32r (TF32-like, full PE rate for free dims >= 256,
~1.5e-4 relative error). Host pre-transposes inputs so the device layout is
transpose-free. exp runs on ACT (the bottleneck engine, ~1 elem/cycle/lane)
in 1024-wide ops; head 0's S^T+exp work for the first n-chunk is emitted
inside the K/V loop so ACT starts as early as possible.
"""

import sys

if "/opt/trn_rl_repo" not in sys.path:
    sys.path.insert(0, "/opt/trn_rl_repo")

import numpy as np

import concourse.tile as tile
from concourse import bacc, mybir
from concourse import bass_utils

P = 128
NTOK = 2048            # n = m = token count per batch element
DIM = 1024
HPC = 4                # heads per core
DH = 64                # head dim
HD = HPC * DH          # 256 = per-core projection width
ECH = DIM // P         # 8 contraction chunks
NCH = NTOK // 512      # 4 n-chunks of 512
MT = NTOK // P         # 16 m-tiles of 128
SCALE = DH ** -0.5
F32 = mybir.dt.float32
F32R = mybir.dt.float32r

_CACHE: dict = {}


def _build(trace_sim: bool = False, repeat: int = 1):
    EXP = mybir.ActivationFunctionType.Exp
    nc = bacc.Bacc("TRN2", target_bir_lowering=False, debug=False, num_devices=8)
    x1T = nc.dram_tensor("x1t", [DIM, NTOK], F32R, kind="ExternalInput").ap()
    x2T = nc.dram_tensor("x2t", [DIM, NTOK], F32R, kind="ExternalInput").ap()
    wqT = nc.dram_tensor("wqt", [DIM, HD], F32R, kind="ExternalInput").ap()
    wkT = nc.dram_tensor("wkt", [DIM, HD], F32R, kind="ExternalInput").ap()
    wvT = nc.dram_tensor("wvt", [DIM, HD], F32R, kind="ExternalInput").ap()
    woT = nc.dram_tensor("wot", [HD, DIM], F32R, kind="ExternalInput").ap()
    y = nc.dram_tensor("y", [NTOK, DIM], F32, kind="ExternalOutput").ap()

    x1T_s = x1T.rearrange("(po pi) n -> pi po n", pi=P)      # [128, 8, 2048]
    x2T_s = x2T.rearrange("(po pi) n -> pi po n", pi=P)
    wqT_r = wqT.rearrange("(po pi) m -> pi po m", pi=P)      # [128, 8, 256]
    wkT_r = wkT.rearrange("(po pi) m -> pi po m", pi=P)
    wvT_r = wvT.rearrange("(po pi) m -> pi po m", pi=P)
    woT_r = woT.rearrange("(po pi) e -> pi po e", pi=P)      # [128, 2, 1024]

    with tile.TileContext(nc, trace_sim=trace_sim) as tc:
      for _rep in range(repeat):
        with (
            tc.tile_pool(name="persist", bufs=1) as persist,
            tc.tile_pool(name="ps1", bufs=4, space="PSUM") as ps1,   # [128,512]
            tc.tile_pool(name="psS", bufs=2, space="PSUM") as psSp,  # [128,1024]
            tc.tile_pool(name="xq", bufs=2) as xqpool,
        ):
            wq_sb = persist.tile([P, ECH, HD], F32R, tag="wq")
            wo_sb = persist.tile([P, 2, DIM], F32R, tag="wo")
            onesf = persist.tile([P, 64], F32, tag="onesf")
            nc.vector.memset(onesf[:], 1.0)
            QT_sb = persist.tile([P, 2, NTOK], F32R, tag="QT")
            O_sb = persist.tile([P, 2, NTOK], F32R, tag="O")
            KT_sb = persist.tile([P, 2, NTOK], F32R, tag="KT")
            V_sb = persist.tile([P, MT, HPC, 65], F32R, tag="V")
            nc.vector.tensor_copy(
                V_sb[:, :, :, 64:65],
                onesf[:].rearrange("p (a b c) -> p a b c", a=MT, b=HPC, c=1),
            )

            def q_proj(nq):
                # Q^T projection for one n-chunk (256-wide x sub-chunks)
                for half in range(2):
                    cs = slice(nq * 512 + half * 256, nq * 512 + half * 256 + 256)
                    xq = xqpool.tile([P, ECH, 256], F32R, tag="xq")
                    for ec in range(ECH):
                        nc.sync.dma_start(xq[:, ec], x1T_s[:, ec, cs])
                    for pg in range(2):
                        psq = ps1.tile(
                            [P, 512], F32, tag="b1", name=f"psq{nq}{half}{pg}"
                        )
                        for ec in range(ECH):
                            nc.tensor.matmul(
                                psq[:, 0:256],
                                wq_sb[:, ec, pg * P:(pg + 1) * P],
                                xq[:, ec, :],
                                start=(ec == 0),
                                stop=(ec == ECH - 1),
                            )
                        nc.vector.tensor_copy(QT_sb[:, pg, cs], psq[:, 0:256])

            def s_exp_pair(nq, h, mtp, expS):
                # one [128,1024] psS pair: S^T for m-tiles (2*mtp, 2*mtp+1)
                pg, off = h // 2, 64 * (h % 2)
                ns = slice(nq * 512, (nq + 1) * 512)
                psS = psSp.tile([P, 1024], F32, tag="psS", name=f"psS{nq}{h}{mtp}")
                for sub in range(2):
                    mt = 2 * mtp + sub
                    nc.tensor.matmul(
                        psS[:, sub * 512:(sub + 1) * 512],
                        KT_sb[off:off + 64, pg, mt * P:(mt + 1) * P],
                        QT_sb[off:off + 64, pg, ns],
                        start=True,
                        stop=True,
                    )
                nc.scalar.activation(
                    expS[:, 2 * mtp:2 * mtp + 2, :].rearrange("p a b -> p (a b)"),
                    psS[:],
                    EXP,
                    scale=SCALE,
                )

            with (
                tc.tile_pool(name="wkv", bufs=1) as wkvpool,
                tc.tile_pool(name="xk", bufs=2) as xkpool,
            ):
                # weights for K first (needed earliest), per-chunk DMAs
                wk_sb = wkvpool.tile([P, ECH, HD], F32R, tag="wk")
                for ec in range(ECH):
                    nc.sync.dma_start(wk_sb[:, ec], wkT_r[:, ec])
                wv_sb = wkvpool.tile([P, ECH, HD], F32R, tag="wv")

                # ---- single x2 pass: K^T projection + V projection; h0's
                # S^T+exp for the first n-chunk is emitted as K tiles land so
                # the ACT engine (bottleneck) starts early ----
                for nq in range(NCH):
                    ns = slice(nq * 512, (nq + 1) * 512)
                    xk = xkpool.tile([P, ECH, 512], F32R, tag="xk")
                    for ec in range(ECH):
                        nc.sync.dma_start(xk[:, ec], x2T_s[:, ec, ns])
                    if nq == 0:
                        for ec in range(ECH):
                            nc.sync.dma_start(wv_sb[:, ec], wvT_r[:, ec])
                    for pg in range(2):
                        psq = ps1.tile([P, 512], F32, tag="b1", name=f"psk{nq}{pg}")
                        for ec in range(ECH):
                            nc.tensor.matmul(
                                psq[:],
                                wk_sb[:, ec, pg * P:(pg + 1) * P],
                                xk[:, ec, :],
                                start=(ec == 0),
                                stop=(ec == ECH - 1),
                            )
                        nc.vector.tensor_copy(KT_sb[:, pg, ns], psq[:])
                    # V for the 4 m-tiles covered by this x2 chunk
                    for sub in range(4):
                        mt = 4 * nq + sub
                        pv = ps1.tile([P, 512], F32, tag="b1", name=f"psv{mt}")
                        for ec in range(ECH):
                            nc.tensor.matmul(
                                pv[:, 0:256],
                                xk[:, ec, sub * P:(sub + 1) * P],
                                wv_sb[:, ec, :],
                                start=(ec == 0),
                                stop=(ec == ECH - 1),
                            )
                        nc.vector.tensor_copy(
                            V_sb[:, mt, :, 0:64],
                            pv[:, 0:256].rearrange("p (h d) -> p h d", d=64),
                        )
                    if nq == 0:
                        nc.sync.dma_start(wq_sb[:], wqT_r)
                        q_proj(0)
                    if nq == NCH - 1:
                        nc.sync.dma_start(wo_sb[:], woT_r)

            # ---- per n-chunk: Q^T projection, attention, out-projection ----
            with (
                tc.tile_pool(name="exps", bufs=2) as expool,
                tc.tile_pool(name="rec", bufs=1) as recpool,
                tc.tile_pool(name="bcp", bufs=1) as bcpool,
                tc.tile_pool(name="otmp", bufs=1) as tmppool,
                tc.tile_pool(name="ysb", bufs=2) as ypool,
            ):
                for nq in range(NCH):
                    ns = slice(nq * 512, (nq + 1) * 512)
                    if nq + 1 < NCH:
                        q_proj(nq + 1)
                    for h in range(HPC):
                        pg, off = h // 2, 64 * (h % 2)
                        expS = expool.tile(
                            [P, MT, 512], F32R, tag="expS", name=f"expS{nq}{h}"
                        )
                        for mtp in range(MT // 2):
                            s_exp_pair(nq, h, mtp, expS)
                        psO = ps1.tile([P, 512], F32, tag="b1", name=f"psO{nq}{h}")
                        for mt in range(MT):
                            nc.tensor.matmul(
                                psO[0:65, :],
                                V_sb[:, mt, h, :],
                                expS[:, mt, :],
                                start=(mt == 0),
                                stop=(mt == MT - 1),
                            )
                        rec = recpool.tile([P, 512], F32R, tag="rec")
                        with nc.allow_low_precision(
                            reason="fp32r rounding of softmax denom reciprocal"
                        ):
                            nc.vector.reciprocal(rec[64:65, :], psO[64:65, :])
                        nc.sync.dma_start(rec[0:1, :], rec[64:65, :])
                        bc = bcpool.tile([64, 512], F32R, tag="bc")
                        nc.gpsimd.partition_broadcast(bc[:], rec[0:1, :])
                        if off == 0:
                            nc.vector.tensor_mul(
                                O_sb[0:64, pg, ns], psO[0:64, :], bc[:]
                            )
                        else:
                            tmp = tmppool.tile([64, 512], F32R, tag="otmp")
                            nc.vector.tensor_mul(tmp[:], psO[0:64, :], bc[:])
                            nc.sync.dma_start(O_sb[64:128, pg, ns], tmp[:])
                    # out-projection deferred one chunk: fills PE gaps while
                    # ACT works on the next chunk's exp
                    if nq > 0:
                        out_proj(nc, nq - 1, ps1, ypool, wo_sb, O_sb, y)
                out_proj(nc, NCH - 1, ps1, ypool, wo_sb, O_sb, y)
    nc.compile()
    return nc


def out_proj(nc, nq, ps1, ypool, wo_sb, O_sb, y):
    for nt in range(4 * nq, 4 * nq + 4):
        y_sb = ypool.tile([P, DIM], F32, tag="y", name=f"ysb{nt}")
        for eo in range(2):
            psY = ps1.tile([P, 512], F32, tag="b1", name=f"psY{nt}{eo}")
            for hd in range(2):
                nc.tensor.matmul(
                    psY[:],
                    O_sb[:, hd, nt * P:(nt + 1) * P],
                    wo_sb[:, hd, eo * 512:(eo + 1) * 512],
                    start=(hd == 0),
                    stop=(hd == 1),
                )
            nc.vector.tensor_copy(y_sb[:, eo * 512:(eo + 1) * 512], psY[:])
        nc.gpsimd.dma_start(y[nt * P:(nt + 1) * P, :], y_sb[:])


def get_nc(trace_sim: bool = False, repeat: int = 1):
    key = ("nc", trace_sim, repeat)
    if key not in _CACHE:
        _CACHE[key] = _build(trace_sim, repeat)
    return _CACHE[key]


def make_in_maps(x1, x2, wq, wk, wv, wo):
    x1 = np.asarray(x1, dtype=np.float32)
    x2 = np.asarray(x2, dtype=np.float32)
    wq = np.asarray(wq, dtype=np.float32)
    wk = np.asarray(wk, dtype=np.float32)
    wv = np.asarray(wv, dtype=np.float32)
    wo = np.asarray(wo, dtype=np.float32)
    in_maps = []
    for core in range(8):
        be, g = core // 4, core % 4
        sl = slice(HD * g, HD * (g + 1))
        in_maps.append({
            "x1t": np.ascontiguousarray(x1[be].T),
            "x2t": np.ascontiguousarray(x2[be].T),
            "wqt": np.ascontiguousarray(wq[sl, :].T),
            "wkt": np.ascontiguousarray(wk[sl, :].T),
            "wvt": np.ascontiguousarray(wv[sl, :].T),
            "wot": np.ascontiguousarray(wo[:, sl].T),
        })
    return in_maps


def assemble(results, bo):
    bo = np.asarray(bo, dtype=np.float32)
    out = np.empty((2, NTOK, DIM), np.float32)
    for be in range(2):
        acc = results[be * 4]["y"].copy()
        for g in range(1, 4):
            acc += results[be * 4 + g]["y"]
        out[be] = acc + bo
    return out


def kernel(x1, x2, wq, wk, wv, wo, bo):
    nc = get_nc()
    in_maps = make_in_maps(x1, x2, wq, wk, wv, wo)
    last_err = None
    for attempt in range(3):
        try:
            res = bass_utils.run_bass_kernel_spmd(
                nc, in_maps, core_ids=list(range(8))
            )
            return assemble(res.results, bo)
        except Exception as e:  # transient NRT_EXEC_UNIT_UNRECOVERABLE etc.
            last_err = e
            import time as _time
            _time.sleep(5 * (attempt + 1))
    raise last_err
